# revision 18
# baseline (speedup 1.0000x reference)
"""GAT (2-layer, PyG-style) on 8 Trainium2 NeuronCores via Bass/Tile. v2.

Differences vs the f32 baseline (same dst-sharded windowed-edge structure):
  - Everything bf16 on the PE and in the gather tables (4x matmul rate,
    2x DMA bytes). PSUM accumulation stays f32.
  - Rotated feature blocks: host builds per-head orthonormal-ish blocks
    B_h with column 0 = a_src1[h], table rows hold h@B (256 bf16 = 512B)
    so alpha_src is just column h*32 of the gathered row -- no separate
    aS gather and rows are exactly 512B. Post-scatter unmix by B^-1
    (2 matmuls/window). Same trick for layer 2 (M2, col 0 = a_src2).
  - alpha_dst per edge via a transposed one-hot T (dst-row-major) built
    on DVE from a DMA-broadcast d128T row, then per-chunk matmuls
    T_k @ aD_win -- removes the 256B-per-edge aD gather entirely
    (one third of baseline gather indices and SWDGE descgen time).
  - Layer-2 rows [rot-h2 (64) | pad] bf16 = 256B with a_src2 folded in;
    aD2 via the same T trick.
Gathers per window drop from 3 to 2 (lo/hi src half of the int16 index
space), gather bytes per edge from 1536B+ to 512B (L1) + 256B (L2).
"""
import sys

for _p in ("/opt/trn_rl_repo", "/opt/pypackages"):
    if _p not in sys.path:
        sys.path.insert(0, _p)

import numpy as np
from concourse import bacc, bass, mybir, tile
from concourse.masks import make_identity

P = 128
F32 = mybir.dt.float32
BF16 = mybir.dt.bfloat16
I16 = mybir.dt.int16
HALF = 32768

# ---- problem constants (nn_GAT_60000693125135) ----
N = 50000
IN_DIM = 256
H1 = 8
HID = 32
HC1 = H1 * HID  # 256
OUT = 64
NCORES = 8
NEG_SLOPE = 0.2

SP = False       # dma_gather single_packet
PMUL4D = True    # one 4D DVE op for the per-head p multiply


def _cdiv(a, b):
    return -(-a // b)


def _wrap16(vals, nidx):
    a = np.asarray(vals, np.int16).reshape(nidx // 16, 16).T
    return np.tile(a, (8, 1))


def _bf16(a):
    import ml_dtypes
    return np.asarray(a, np.float32).astype(ml_dtypes.bfloat16)


# ----------------------------------------------------------------------------
# Host-side preprocessing.
# ----------------------------------------------------------------------------
def prep_edges(edge_index, n, ncores):
    """Shard + window + src-half-split the edge list (self loops added).

    Per-core arrays: srclo16/srchi16 (wrap-16 gather idx), d128 [P, CT]
    (dst row in window, -1 on pads), d128T [nw, cmax*128] (transposed
    layout for the T build, -1 on pads)."""
    e0 = edge_index[0].astype(np.int64)
    e1 = edge_index[1].astype(np.int64)
    loops = np.arange(n, dtype=np.int64)
    src = np.concatenate([e0, loops])
    dst = np.concatenate([e1, loops])

    nchunk = n // ncores
    nw = _cdiv(nchunk, P)
    core = dst // nchunk
    dloc = dst - core * nchunk
    w = dloc // P
    hi = (src >= HALF).astype(np.int64)
    gid = (core * nw + w) * 2 + hi
    ngroups = ncores * nw * 2
    cnt = np.bincount(gid, minlength=ngroups).reshape(ncores, nw, 2)
    CWlo = _cdiv(cnt[:, :, 0].max(axis=0), P)
    CWhi = _cdiv(cnt[:, :, 1].max(axis=0), P)
    CW = CWlo + CWhi
    assert CW.min() >= 1
    CTlo, CThi, CT = int(CWlo.sum()), int(CWhi.sum()), int(CW.sum())
    cmax = int(CW.max())

    order = np.argsort(gid, kind="stable")
    gid_s = gid[order]
    starts = np.concatenate([[0], np.cumsum(np.bincount(gid_s, minlength=ngroups))])
    pos = np.arange(order.size) - starts[gid_s]
    c_s = core[order]
    w_s = w[order]
    hi_s = hi[order]
    slot = pos + hi_s * (CWlo[w_s] * P)

    srcv = np.zeros((ncores, nw, cmax * P), np.int64)
    d128v = np.full((ncores, nw, cmax * P), -1.0, np.float32)
    srcv[c_s, w_s, slot] = src[order] - hi_s * HALF
    d128v[c_s, w_s, slot] = (dloc[order] % P).astype(np.float32)

    srclo16 = np.zeros((ncores, P, 8 * CTlo), np.int16)
    srchi16 = np.zeros((ncores, P, 8 * CThi), np.int16)
    d128A = np.full((ncores, P, CT), -1.0, np.float32)
    d128T = np.full((ncores, nw, cmax * P), -1.0, np.float32)
    olo = ohi = oall = 0
    for wi in range(nw):
        nlo, nhi, nall = int(CWlo[wi]) * P, int(CWhi[wi]) * P, int(CW[wi]) * P
        for c in range(ncores):
            if nlo:
                srclo16[c, :, 8 * olo:8 * (olo + nlo // P)] = _wrap16(srcv[c, wi, :nlo], nlo)
            if nhi:
                srchi16[c, :, 8 * ohi:8 * (ohi + nhi // P)] = _wrap16(srcv[c, wi, nlo:nall], nhi)
            d128A[c, :, oall:oall + nall // P] = d128v[c, wi, :nall].reshape(nall // P, P).T
            d128T[c, wi, :nall] = d128v[c, wi, :nall]
        olo += nlo // P
        ohi += nhi // P
        oall += nall // P
    return dict(srclo16=srclo16, srchi16=srchi16, d128=d128A, d128T=d128T,
                CWlo=[int(v) for v in CWlo], CWhi=[int(v) for v in CWhi],
                cmax=cmax)


def _rot(a):
    """Invertible [d, d] block with column 0 == a, rest orthonormal."""
    a = np.asarray(a, np.float64)
    d = a.size
    q, _ = np.linalg.qr(np.column_stack([a, np.eye(d)[:, 1:]]))
    B = q.copy()
    B[:, 0] = a
    s = float(q[:, 0] @ a)
    Binv = q.T.copy()
    Binv[0, :] /= s
    assert np.abs(B @ Binv - np.eye(d)).max() < 1e-9
    return B.astype(np.float64), Binv.astype(np.float64)


# ----------------------------------------------------------------------------
# Kernel builder (SPMD program, same for all cores).
# ----------------------------------------------------------------------------
def build_nc(cfg):
    n = cfg["N"]; in_dim = cfg["IN"]; hc1 = cfg["HC1"]; h1 = cfg["H1"]
    out_dim = cfg["OUT"]; ncores = cfg["NCORES"]; neg = cfg["NEG"]
    CWlo, CWhi = cfg["CWlo"], cfg["CWhi"]
    CW = [a + b for a, b in zip(CWlo, CWhi)]
    cmax = cfg["cmax"]
    b1nz, b2nz = cfg["B1NZ"], cfg["B2NZ"]

    nchunk = n // ncores
    nw = _cdiv(nchunk, P)
    assert len(CW) == nw
    CTlo, CThi, CT = sum(CWlo), sum(CWhi), sum(CW)
    ntiles = _cdiv(n, P)
    npad = ntiles * P
    nlpad = nw * P
    kt1 = _cdiv(in_dim, P)   # 2
    ckt = _cdiv(hc1, P)      # 2
    NB = 8
    W2C = out_dim + 1        # 65: [rot-h2 | aD2]

    CMAXP = cmax * P
    nc = bacc.Bacc(None, target_bir_lowering=False, debug=False,
                   num_devices=ncores, num_swdge_queues=4,
                   dynamic_dma_scratch_size=24576)

    # ---- I/O ----
    xT_in = nc.dram_tensor("xT", [in_dim, npad], BF16, kind="ExternalInput")
    w1e_in = nc.dram_tensor("W1e", [in_dim, hc1 + h1], BF16, kind="ExternalInput")
    binv_in = nc.dram_tensor("Binv", [hc1, hc1], BF16, kind="ExternalInput")
    w2e_in = nc.dram_tensor("W2e", [hc1, W2C], BF16, kind="ExternalInput")
    m2i_in = nc.dram_tensor("M2i", [out_dim, out_dim], BF16, kind="ExternalInput")
    b1r_in = nc.dram_tensor("b1r", [P, hc1], BF16, kind="ExternalInput")
    b2r_in = nc.dram_tensor("b2r", [P, out_dim], F32, kind="ExternalInput")
    iota_in = nc.dram_tensor("iota", [P, P], BF16, kind="ExternalInput")
    iotac_in = nc.dram_tensor("iotac", [P, 1], F32, kind="ExternalInput")
    ones_in = nc.dram_tensor("ones1", [1, P], BF16, kind="ExternalInput")
    d128_in = nc.dram_tensor("d128", [P, CT], BF16, kind="ExternalInput")
    d128T_in = nc.dram_tensor("d128T", [nw, cmax * P], BF16, kind="ExternalInput")
    slo_in = nc.dram_tensor("srclo16", [P, 8 * CTlo], I16, kind="ExternalInput")
    shi_in = nc.dram_tensor("srchi16", [P, max(8 * CThi, 16)], I16, kind="ExternalInput")
    out_ext = nc.dram_tensor("out", [nchunk, out_dim], F32, kind="ExternalOutput")

    from contextlib import ExitStack
    with tile.TileContext(nc) as tc, ExitStack() as es:
        if True:
            dram = es.enter_context(tc.tile_pool(name="dram", bufs=1, space="DRAM"))
            cpool = es.enter_context(tc.tile_pool(name="const", bufs=1))
            xpool = es.enter_context(tc.tile_pool(name="xst", bufs=2))
            hpool = es.enter_context(tc.tile_pool(name="hst", bufs=2))
            gpool = es.enter_context(tc.tile_pool(name="gbuf", bufs=5))
            gwpool = es.enter_context(tc.tile_pool(name="gw", bufs=2))
            g2pool = es.enter_context(tc.tile_pool(name="g2buf", bufs=4))
            spool = es.enter_context(tc.tile_pool(name="sbuf", bufs=2))
            tpool = es.enter_context(tc.tile_pool(name="tbuf", bufs=2))
            pepool = es.enter_context(tc.tile_pool(name="pebuf", bufs=2))
            o1pool = es.enter_context(tc.tile_pool(name="o1", bufs=2))
            smpool = es.enter_context(tc.tile_pool(name="small", bufs=3))
            psA = es.enter_context(tc.tile_pool(name="psA", bufs=2, space="PSUM"))
            psB = es.enter_context(tc.tile_pool(name="psB", bufs=1, space="PSUM"))
            psD = es.enter_context(tc.tile_pool(name="psD", bufs=2, space="PSUM"))
            psT = es.enter_context(tc.tile_pool(name="psT", bufs=1, space="PSUM"))
            psQ = es.enter_context(tc.tile_pool(name="psQ", bufs=1, space="PSUM"))

            # ---- DRAM scratch ----
            h_tab = dram.tile([npad, hc1], BF16)
            aDfull = dram.tile([npad, h1], BF16)
            h2_mine = dram.tile([nchunk, out_dim], BF16)
            aD2_loc = dram.tile([nlpad, 1], BF16)
            h2c_tab = dram.tile([n, out_dim], BF16, addr_space="Shared")
            h2_tab = dram.tile([n, 2 * out_dim], BF16)

            # ---- resident constants ----
            iota_t = cpool.tile([P, P], BF16)
            nc.sync.dma_start(out=iota_t[:], in_=iota_in[:])
            iotac_t = cpool.tile([P, 1], F32)
            nc.sync.dma_start(out=iotac_t[:], in_=iotac_in[:])
            ones1 = cpool.tile([1, P], BF16)
            nc.sync.dma_start(out=ones1[:], in_=ones_in[:])
            d128_t = cpool.tile([P, CT], BF16)
            nc.sync.dma_start(out=d128_t[:], in_=d128_in[:])
            ident = cpool.tile([P, P], BF16)
            make_identity(nc, ident[:])
            b1r = cpool.tile([P, hc1], BF16)
            nc.sync.dma_start(out=b1r[:], in_=b1r_in[:])
            b2r = cpool.tile([P, out_dim], F32)
            nc.sync.dma_start(out=b2r[:], in_=b2r_in[:])
            slo = cpool.tile([P, 8 * CTlo], I16)
            nc.sync.dma_start(out=slo[:], in_=slo_in[:])
            shi = cpool.tile([P, max(8 * CThi, 16)], I16)
            nc.sync.dma_start(out=shi[:], in_=shi_in[:])
            w1e = cpool.tile([P, kt1, hc1 + h1], BF16)
            for kt in range(kt1):
                nc.sync.dma_start(out=w1e[:, kt, :], in_=w1e_in[kt * P:(kt + 1) * P, :])
            binv = cpool.tile([P, ckt, hc1], BF16)
            for c in range(ckt):
                nc.sync.dma_start(out=binv[:, c, :], in_=binv_in[c * P:(c + 1) * P, :])
            w2e = cpool.tile([P, ckt, W2C], BF16)
            for c in range(ckt):
                nc.sync.dma_start(out=w2e[:, c, :], in_=w2e_in[c * P:(c + 1) * P, :])
            m2i = cpool.tile([out_dim, out_dim], BF16)
            nc.sync.dma_start(out=m2i[:], in_=m2i_in[:])

            # ---- phase 1: h_ext = x @ w1e -> h_tab + aDfull (replicated) ----
            for g in range(_cdiv(ntiles, NB)):
                nt0 = g * NB
                nb = min(NB, ntiles - nt0)
                xst = xpool.tile([P, kt1, NB * P], BF16, tag="xst")
                for kt in range(kt1):
                    nc.sync.dma_start(out=xst[:, kt, 0:nb * P],
                                      in_=xT_in[kt * P:(kt + 1) * P, nt0 * P:(nt0 + nb) * P])
                hstg = hpool.tile([P, NB, hc1 + h1], BF16, tag="hst")
                for j in range(nb):
                    ps = psA.tile([P, hc1 + h1], F32, tag="ops")
                    for kt in range(kt1):
                        nc.tensor.matmul(out=ps[:], lhsT=xst[:, kt, j * P:(j + 1) * P],
                                         rhs=w1e[:, kt, :], start=(kt == 0), stop=(kt == kt1 - 1))
                    nc.scalar.copy(out=hstg[:, j, :], in_=ps[:])
                hv = h_tab[nt0 * P:(nt0 + nb) * P, :].rearrange("(j p) c -> p j c", p=P)
                nc.sync.dma_start(out=hv, in_=hstg[:, 0:nb, 0:hc1])
                av = aDfull[nt0 * P:(nt0 + nb) * P, :].rearrange("(j p) c -> p j c", p=P)
                nc.sync.dma_start(out=av, in_=hstg[:, 0:nb, hc1:])

            pid_rows = nc.sync.snap(nc.sync.partition_id() * nchunk)
            stop = cfg.get("STOP", "")

            def bounce_out(src_dram, width):
                for w in range(nw):
                    rows = min(P, nchunk - w * P)
                    dbg = smpool.tile([P, out_dim], F32, tag="dbg")
                    nc.vector.tensor_scalar(
                        out=dbg[:rows, :],
                        in0=src_dram[w * P:w * P + rows, 0:width],
                        scalar1=1.0, scalar2=None, op0=mybir.AluOpType.mult)
                    nc.sync.dma_start(out=out_ext[w * P:w * P + rows, :],
                                      in_=dbg[:rows, :])

            if stop == "phase1":
                # bounce own-chunk h_tab rows (rotated) for host check
                for w in range(nw):
                    rows = min(P, nchunk - w * P)
                    dbg = smpool.tile([P, out_dim], F32, tag="dbg")
                    src = h_tab[bass.ds(pid_rows + w * P, rows), 0:out_dim]
                    sb = smpool.tile([P, out_dim], BF16, tag="dbgb")
                    nc.sync.dma_start(out=sb[:rows, :], in_=src)
                    nc.vector.tensor_scalar(out=dbg[:rows, :], in0=sb[:rows, :],
                                            scalar1=1.0, scalar2=None,
                                            op0=mybir.AluOpType.mult)
                    nc.sync.dma_start(out=out_ext[w * P:w * P + rows, :],
                                      in_=dbg[:rows, :])
                return nc

            # ---- phase 2: layer-1 edge aggregation per dst window ----
            # software-pipelined: stage A(w+1) is emitted before stage B(w)
            # so each engine always has independent work queued.
            OLO = [0] * nw
            OALL = [0] * nw
            _olo = _oall = 0
            for w in range(nw):
                OLO[w] = _olo; OALL[w] = _oall
                _olo += CWlo[w]; _oall += CW[w]

            def p2_stageA(w):
                Clo, Chi, C = CWlo[w], CWhi[w], CW[w]
                rows = min(P, nchunk - w * P)
                olo, oall = OLO[w], OALL[w]
                G = gpool.tile([P, cmax, hc1], BF16, tag="G")
                if Clo:
                    nc.gpsimd.dma_gather(
                        out_ap=G[:, 0:Clo, :], in_ap=h_tab[0:HALF, :],
                        idxs_ap=slo[:, 8 * olo:8 * (olo + Clo)],
                        num_idxs=Clo * P, num_idxs_reg=Clo * P, elem_size=hc1,
                        single_packet=SP, queue_num=(2 * w) % 4)
                if Chi:
                    nc.gpsimd.dma_gather(
                        out_ap=G[:, Clo:C, :], in_ap=h_tab[HALF:, :],
                        idxs_ap=shi[:, 8 * (oall - olo):8 * (oall - olo + Chi)],
                        num_idxs=Chi * P, num_idxs_reg=Chi * P, elem_size=hc1,
                        single_packet=SP, queue_num=(2 * w + 1) % 4)
                aDw = smpool.tile([P, h1], BF16, tag="aDw")
                nc.sync.dma_start(out=aDw[:rows, :],
                                  in_=aDfull[bass.ds(pid_rows + w * P, rows), :])
                S = spool.tile([P, cmax, P], BF16, tag="S")
                nc.vector.tensor_tensor(
                    out=S[:, 0:C, :],
                    in0=d128_t[:, oall:oall + C].unsqueeze(-1).to_broadcast((P, C, P)),
                    in1=iota_t[:].unsqueeze(1).to_broadcast((P, C, P)),
                    op=mybir.AluOpType.is_equal)
                dT = tpool.tile([1, cmax * P], BF16, tag="dT")
                nc.sync.dma_start(out=dT[:, 0:C * P], in_=d128T_in[w:w + 1, 0:C * P])
                T = tpool.tile([P, cmax, P], BF16, tag="T")
                for g0 in range(0, C, 8):
                    gn = min(8, C - g0)
                    Qps = psQ.tile([P, 8 * P], F32, tag="q")
                    for h0 in range(0, gn, 4):
                        hn = min(4, gn - h0)
                        nc.tensor.matmul(
                            out=Qps[:, h0 * P:(h0 + hn) * P], lhsT=ones1[:],
                            rhs=dT[0:1, (g0 + h0) * P:(g0 + h0 + hn) * P],
                            start=True, stop=True)
                    nc.vector.tensor_tensor(
                        out=T[:, g0:g0 + gn, :],
                        in0=Qps[:, 0:gn * P].rearrange("r (k q) -> r k q", k=gn),
                        in1=iotac_t[:, 0:1].unsqueeze(1).to_broadcast((P, gn, P)),
                        op=mybir.AluOpType.is_equal)
                aDps = psD.tile([P, cmax, h1], F32, tag="aD")
                for k in range(C):
                    nc.tensor.matmul(out=aDps[:, k, :], lhsT=T[:, k, :], rhs=aDw[:],
                                     start=True, stop=True)
                aDsb = smpool.tile([P, cmax, h1], BF16, tag="aDsb")
                nc.scalar.copy(out=aDsb[:, 0:C, :], in_=aDps[:, 0:C, :])
                pe = pepool.tile([P, cmax, h1], BF16, tag="pe")
                G4 = G[:, 0:C, :].rearrange("p c (h j) -> p c h j", h=h1)
                nc.vector.tensor_tensor(
                    out=pe[:, 0:C, :], in0=G4[:, :, :, 0].squeeze(),
                    in1=aDsb[:, 0:C, :], op=mybir.AluOpType.add)
                nc.vector.scalar_tensor_tensor(
                    out=pe[:, 0:C, :], in0=pe[:, 0:C, :], scalar=neg,
                    in1=pe[:, 0:C, :], op0=mybir.AluOpType.mult,
                    op1=mybir.AluOpType.max)
                nc.scalar.activation(out=pe[:, 0:C, :], in_=pe[:, 0:C, :],
                                     func=mybir.ActivationFunctionType.Exp)
                GW = gwpool.tile([P, cmax, hc1 + h1], BF16, tag="GW")
                GW4 = GW[:, 0:C, 0:hc1].rearrange("p c (h j) -> p c h j", h=h1)
                nc.vector.tensor_tensor(
                    out=GW4, in0=G4,
                    in1=pe[:, 0:C, :].unsqueeze(-1).to_broadcast((P, C, h1, HID)),
                    op=mybir.AluOpType.mult)
                nc.scalar.copy(out=GW[:, 0:C, hc1:], in_=pe[:, 0:C, :])
                return S, GW

            def p2_stageB(w, S, GW):
                Clo, Chi, C = CWlo[w], CWhi[w], CW[w]
                rows = min(P, nchunk - w * P)
                ops = psA.tile([P, hc1 + h1], F32, tag="ops")
                for k in range(C):
                    nc.tensor.matmul(out=ops[:], lhsT=S[:, k, :], rhs=GW[:, k, :],
                                     start=(k == 0), stop=(k == C - 1))
                rec = smpool.tile([P, h1], F32, tag="rec")
                nc.vector.reciprocal(out=rec[:], in_=ops[:, hc1:])
                opssb = o1pool.tile([P, hc1], BF16, tag="opssb")
                nc.scalar.copy(out=opssb[:], in_=ops[:, 0:hc1])
                ats = []
                for c in range(ckt):
                    tp = psT.tile([P, P], BF16, tag="tp")
                    nc.tensor.transpose(tp[:], opssb[:, c * P:(c + 1) * P], ident[:])
                    at = o1pool.tile([P, P], BF16, tag="at")
                    nc.scalar.copy(out=at[:], in_=tp[:])
                    ats.append(at)
                h1u = psB.tile([P, hc1], F32, tag="h1u")
                for c in range(ckt):
                    nc.tensor.matmul(out=h1u[:], lhsT=ats[c][:], rhs=binv[:, c, :],
                                     start=(c == 0), stop=(c == ckt - 1))
                h1w = o1pool.tile([P, hc1], BF16, tag="h1w")
                nc.vector.tensor_tensor(
                    out=h1w[:].rearrange("p (h j) -> p h j", h=h1),
                    in0=h1u[:].rearrange("p (h j) -> p h j", h=h1),
                    in1=rec[:].unsqueeze(-1).to_broadcast((P, h1, HID)),
                    op=mybir.AluOpType.mult)
                if b1nz:
                    nc.vector.tensor_tensor(out=h1w[:], in0=h1w[:], in1=b1r[:],
                                            op=mybir.AluOpType.add)
                nc.scalar.activation(out=h1w[:], in_=h1w[:],
                                     func=mybir.ActivationFunctionType.Relu)
                ats2 = []
                for c in range(ckt):
                    tp = psT.tile([P, P], BF16, tag="tp")
                    nc.tensor.transpose(tp[:], h1w[:, c * P:(c + 1) * P], ident[:])
                    at = o1pool.tile([P, P], BF16, tag="at")
                    nc.scalar.copy(out=at[:], in_=tp[:])
                    ats2.append(at)
                h2e = psB.tile([P, W2C], F32, tag="h1u")
                for c in range(ckt):
                    nc.tensor.matmul(out=h2e[:], lhsT=ats2[c][:], rhs=w2e[:, c, :],
                                     start=(c == 0), stop=(c == ckt - 1))
                h2sb = o1pool.tile([P, out_dim], BF16, tag="h2sb")
                nc.scalar.copy(out=h2sb[:], in_=h2e[:, 0:out_dim])
                nc.sync.dma_start(out=h2_mine[w * P:w * P + rows, :], in_=h2sb[:rows, :])
                a2sb = smpool.tile([P, 1], BF16, tag="a2sb")
                nc.scalar.copy(out=a2sb[:], in_=h2e[:, out_dim:out_dim + 1])
                nc.sync.dma_start(out=aD2_loc[w * P:w * P + rows, :], in_=a2sb[:rows, :])

            prev = None
            for w in range(nw):
                cur = p2_stageA(w)
                if prev is not None:
                    p2_stageB(w - 1, *prev)
                prev = cur
            p2_stageB(nw - 1, *prev)

            if stop == "phase2":
                bounce_out(h2_mine, out_dim)
                return nc

            # ---- all-gather h2 ----
            nc.gpsimd.collective_compute(
                "AllGather", mybir.AluOpType.bypass,
                replica_groups=[list(range(ncores))],
                ins=[h2_mine[:].opt()], outs=[h2c_tab[:].opt()])
            # repad 64 -> 128 byte-stride rows for the 256B-min gather
            NRP = 32
            r0 = 0
            while r0 < n:
                nbr = min(NRP, (n - r0) // P)
                if nbr >= 1:
                    rows2 = nbr * P
                    rp = hpool.tile([P, NRP, out_dim], BF16, tag="rp")
                    nc.sync.dma_start(
                        out=rp[:, 0:nbr, :],
                        in_=h2c_tab[r0:r0 + rows2, :].rearrange(
                            "(j p) c -> p j c", p=P))
                    nc.sync.dma_start(
                        out=h2_tab[r0:r0 + rows2, 0:out_dim].rearrange(
                            "(j p) c -> p j c", p=P),
                        in_=rp[:, 0:nbr, :])
                    r0 += rows2
                else:
                    rem = n - r0
                    rp = hpool.tile([P, NRP, out_dim], BF16, tag="rp")
                    nc.sync.dma_start(out=rp[0:rem, 0, :], in_=h2c_tab[r0:n, :])
                    nc.sync.dma_start(out=h2_tab[r0:n, 0:out_dim],
                                      in_=rp[0:rem, 0, :])
                    r0 = n

            if stop == "cc":
                bounce_out(h2_tab, out_dim)
                return nc

            # ---- phase 3: layer-2 edge aggregation + log_softmax ----
            t_all = cpool.tile([P, nw, out_dim], F32)
            s_all = cpool.tile([P, nw], F32)

            def p3_stageA(w):
                Clo, Chi, C = CWlo[w], CWhi[w], CW[w]
                rows = min(P, nchunk - w * P)
                olo, oall = OLO[w], OALL[w]
                G2 = g2pool.tile([P, cmax, 2 * out_dim], BF16, tag="G2")
                if Clo:
                    nc.gpsimd.dma_gather(
                        out_ap=G2[:, 0:Clo, :], in_ap=h2_tab[0:HALF, :],
                        idxs_ap=slo[:, 8 * olo:8 * (olo + Clo)],
                        num_idxs=Clo * P, num_idxs_reg=Clo * P,
                        elem_size=2 * out_dim, single_packet=SP,
                        queue_num=(2 * w) % 4)
                if Chi:
                    nc.gpsimd.dma_gather(
                        out_ap=G2[:, Clo:C, :], in_ap=h2_tab[HALF:, :],
                        idxs_ap=shi[:, 8 * (oall - olo):8 * (oall - olo + Chi)],
                        num_idxs=Chi * P, num_idxs_reg=Chi * P,
                        elem_size=2 * out_dim, single_packet=SP,
                        queue_num=(2 * w + 1) % 4)
                aD2w = smpool.tile([P, 1], BF16, tag="aD2w")
                nc.sync.dma_start(out=aD2w[:rows, :],
                                  in_=aD2_loc[w * P:w * P + rows, :])
                S = spool.tile([P, cmax, P], BF16, tag="S")
                nc.vector.tensor_tensor(
                    out=S[:, 0:C, :],
                    in0=d128_t[:, oall:oall + C].unsqueeze(-1).to_broadcast((P, C, P)),
                    in1=iota_t[:].unsqueeze(1).to_broadcast((P, C, P)),
                    op=mybir.AluOpType.is_equal)
                dT = tpool.tile([1, cmax * P], BF16, tag="dT")
                nc.sync.dma_start(out=dT[:, 0:C * P], in_=d128T_in[w:w + 1, 0:C * P])
                T = tpool.tile([P, cmax, P], BF16, tag="T")
                for g0 in range(0, C, 8):
                    gn = min(8, C - g0)
                    Qps = psQ.tile([P, 8 * P], F32, tag="q")
                    for h0 in range(0, gn, 4):
                        hn = min(4, gn - h0)
                        nc.tensor.matmul(
                            out=Qps[:, h0 * P:(h0 + hn) * P], lhsT=ones1[:],
                            rhs=dT[0:1, (g0 + h0) * P:(g0 + h0 + hn) * P],
                            start=True, stop=True)
                    nc.vector.tensor_tensor(
                        out=T[:, g0:g0 + gn, :],
                        in0=Qps[:, 0:gn * P].rearrange("r (k q) -> r k q", k=gn),
                        in1=iotac_t[:, 0:1].unsqueeze(1).to_broadcast((P, gn, P)),
                        op=mybir.AluOpType.is_equal)
                aD2ps = psD.tile([P, cmax, 1], F32, tag="aD")
                for k in range(C):
                    nc.tensor.matmul(out=aD2ps[:, k, :], lhsT=T[:, k, :], rhs=aD2w[:],
                                     start=True, stop=True)
                aD2sb = smpool.tile([P, cmax], BF16, tag="aD2sb")
                nc.scalar.copy(out=aD2sb[:, 0:C], in_=aD2ps[:, 0:C, 0].squeeze())
                pe2 = pepool.tile([P, cmax], BF16, tag="pe2")
                nc.vector.tensor_tensor(
                    out=pe2[:, 0:C], in0=G2[:, 0:C, 0].squeeze(),
                    in1=aD2sb[:, 0:C], op=mybir.AluOpType.add)
                nc.vector.scalar_tensor_tensor(
                    out=pe2[:, 0:C], in0=pe2[:, 0:C], scalar=neg,
                    in1=pe2[:, 0:C], op0=mybir.AluOpType.mult,
                    op1=mybir.AluOpType.max)
                nc.scalar.activation(out=pe2[:, 0:C], in_=pe2[:, 0:C],
                                     func=mybir.ActivationFunctionType.Exp)
                G2b = pepool.tile([P, cmax, out_dim + 1], BF16, tag="G2b")
                nc.vector.tensor_tensor(
                    out=G2b[:, 0:C, 0:out_dim], in0=G2[:, 0:C, 0:out_dim],
                    in1=pe2[:, 0:C].unsqueeze(-1).to_broadcast((P, C, out_dim)),
                    op=mybir.AluOpType.mult)
                nc.scalar.copy(out=G2b[:, 0:C, out_dim].squeeze(), in_=pe2[:, 0:C])
                return S, G2b

            def p3_stageB(w, S, G2b):
                Clo, Chi, C = CWlo[w], CWhi[w], CW[w]
                rows = min(P, nchunk - w * P)
                ops2 = psA.tile([P, out_dim + 1], F32, tag="ops")
                for k in range(C):
                    nc.tensor.matmul(out=ops2[:], lhsT=S[:, k, :],
                                     rhs=G2b[:, k, :], start=(k == 0), stop=(k == C - 1))
                rec2 = smpool.tile([P, 1], F32, tag="rec2")
                nc.vector.reciprocal(out=rec2[:], in_=ops2[:, out_dim:])
                o2sb = o1pool.tile([P, out_dim], BF16, tag="o2sb")
                nc.scalar.copy(out=o2sb[:], in_=ops2[:, 0:out_dim])
                tp = psT.tile([P, P], BF16, tag="tp")
                nc.tensor.transpose(tp[0:out_dim, :], o2sb[:], ident[:])
                at5 = o1pool.tile([out_dim, P], BF16, tag="at5")
                nc.scalar.copy(out=at5[:], in_=tp[0:out_dim, :])
                z = psB.tile([P, out_dim], F32, tag="h1u")
                nc.tensor.matmul(out=z[:], lhsT=at5[:], rhs=m2i[:],
                                 start=True, stop=True)
                zf = smpool.tile([P, out_dim], F32, tag="zf")
                nc.vector.tensor_tensor(out=zf[:], in0=z[:],
                                        in1=rec2[:].to_broadcast((P, out_dim)),
                                        op=mybir.AluOpType.mult)
                if b2nz:
                    nc.vector.tensor_tensor(out=zf[:], in0=zf[:], in1=b2r[:],
                                            op=mybir.AluOpType.add)
                negmax = smpool.tile([P, 1], F32, tag="negmax")
                nc.vector.tensor_reduce(out=negmax[:], in_=zf[:],
                                        axis=mybir.AxisListType.X,
                                        op=mybir.AluOpType.max, negate=True)
                nc.vector.tensor_tensor(out=t_all[:, w, :], in0=zf[:],
                                        in1=negmax[:].to_broadcast((P, out_dim)),
                                        op=mybir.AluOpType.add)
                esc = smpool.tile([P, out_dim], F32, tag="esc")
                nc.scalar.activation(out=esc[:], in_=t_all[:, w, :],
                                     func=mybir.ActivationFunctionType.Exp,
                                     accum_out=s_all[:, w:w + 1])

            prev3 = None
            for w in range(nw):
                cur = p3_stageA(w)
                if prev3 is not None:
                    p3_stageB(w - 1, *prev3)
                prev3 = cur
            p3_stageB(nw - 1, *prev3)
            lns = cpool.tile([P, nw], F32)
            nc.scalar.activation(out=lns[:], in_=s_all[:],
                                 func=mybir.ActivationFunctionType.Ln)
            for w in range(nw):
                rows = min(P, nchunk - w * P)
                res = smpool.tile([P, out_dim], F32, tag="esc")
                nc.vector.tensor_tensor(out=res[:], in0=t_all[:, w, :],
                                        in1=lns[:, w:w + 1].to_broadcast((P, out_dim)),
                                        op=mybir.AluOpType.subtract)
                nc.sync.dma_start(out=out_ext[w * P:w * P + rows, :], in_=res[:rows, :])

    return nc


# ----------------------------------------------------------------------------
# Host-side input packing.
# ----------------------------------------------------------------------------
def make_in_maps(inputs, cfg):
    n = cfg["N"]; in_dim = cfg["IN"]; hc1 = cfg["HC1"]; h1 = cfg["H1"]
    hid = cfg["HID"]; out_dim = cfg["OUT"]; ncores = cfg["NCORES"]

    x = np.asarray(inputs["x"], np.float32)
    ei = np.asarray(inputs["edge_index"])
    W1 = np.asarray(inputs["W1"], np.float64)
    a_src1 = np.asarray(inputs["a_src1"], np.float64)
    a_dst1 = np.asarray(inputs["a_dst1"], np.float64)
    b1 = np.asarray(inputs["b1"], np.float32)
    W2 = np.asarray(inputs["W2"], np.float64)
    a_src2 = np.asarray(inputs["a_src2"], np.float64)
    a_dst2 = np.asarray(inputs["a_dst2"], np.float64)
    b2 = np.asarray(inputs["b2"], np.float32)

    cfg["B1NZ"] = bool(np.any(b1))
    cfg["B2NZ"] = bool(np.any(b2))

    ntiles = _cdiv(n, P)
    npad = ntiles * P
    xT = np.zeros((in_dim, npad), np.float32)
    xT[:, :n] = x.T

    # rotation blocks: B_h col 0 = a_src1[h]
    W1e = np.zeros((in_dim, hc1 + h1), np.float64)
    Binv = np.zeros((hc1, hc1), np.float64)
    for h in range(h1):
        B, Bi = _rot(a_src1[h])
        W1e[:, h * hid:(h + 1) * hid] = W1[:, h * hid:(h + 1) * hid] @ B
        W1e[:, hc1 + h] = W1[:, h * hid:(h + 1) * hid] @ a_dst1[h]
        Binv[h * hid:(h + 1) * hid, h * hid:(h + 1) * hid] = Bi
    M2, M2i = _rot(a_src2[0])
    W2e = np.zeros((hc1, out_dim + 1), np.float64)
    W2e[:, 0:out_dim] = W2 @ M2
    W2e[:, out_dim] = W2 @ a_dst2[0]

    pe = prep_edges(ei, n, ncores)
    cfg["CWlo"], cfg["CWhi"], cfg["cmax"] = pe["CWlo"], pe["CWhi"], pe["cmax"]

    iota = np.tile(np.arange(P, dtype=np.float32)[None, :], (P, 1))
    common = {
        "xT": _bf16(xT),
        "W1e": _bf16(W1e), "Binv": _bf16(Binv),
        "W2e": _bf16(W2e), "M2i": _bf16(M2i),
        "b1r": _bf16(np.tile(b1[None, :], (P, 1))),
        "b2r": np.tile(b2[None, :], (P, 1)).astype(np.float32),
        "iota": _bf16(iota),
        "iotac": np.arange(P, dtype=np.float32)[:, None],
        "ones1": _bf16(np.ones((1, P), np.float32)),
    }
    in_maps = []
    for c in range(ncores):
        m = dict(common)
        m["srclo16"] = np.ascontiguousarray(pe["srclo16"][c])
        shi = pe["srchi16"][c]
        if shi.shape[1] == 0:
            shi = np.zeros((P, 16), np.int16)
        m["srchi16"] = np.ascontiguousarray(shi)
        m["d128"] = _bf16(pe["d128"][c])
        m["d128T"] = _bf16(pe["d128T"][c])
        in_maps.append(m)
    return in_maps


DEFAULT_CFG = dict(N=N, IN=IN_DIM, HC1=HC1, H1=H1, HID=HID, OUT=OUT,
                   NCORES=NCORES, NEG=NEG_SLOPE)

TRACE = False
LAST_RESULTS = None


def kernel(**inputs) -> np.ndarray:
    global LAST_RESULTS
    from concourse.bass_utils import run_bass_kernel_spmd

    cfg = dict(DEFAULT_CFG)
    in_maps = make_in_maps(inputs, cfg)
    nc = build_nc(cfg)
    if not nc.is_finalized():
        nc.finalize()
    res = run_bass_kernel_spmd(nc, in_maps, core_ids=list(range(cfg["NCORES"])),
                               trace=TRACE)
    LAST_RESULTS = res
    outs = [res.results[c]["out"] for c in range(cfg["NCORES"])]
    return np.concatenate(outs, axis=0).astype(np.float32)


# revision 19
# speedup vs baseline: 3.3888x; 3.3888x over previous
"""GAT (2-layer, PyG-style) on 8 Trainium2 NeuronCores via Bass/Tile. v2.

Differences vs the f32 baseline (same dst-sharded windowed-edge structure):
  - Everything bf16 on the PE and in the gather tables (4x matmul rate,
    2x DMA bytes). PSUM accumulation stays f32.
  - Rotated feature blocks: host builds per-head orthonormal-ish blocks
    B_h with column 0 = a_src1[h], table rows hold h@B (256 bf16 = 512B)
    so alpha_src is just column h*32 of the gathered row -- no separate
    aS gather and rows are exactly 512B. Post-scatter unmix by B^-1
    (2 matmuls/window). Same trick for layer 2 (M2, col 0 = a_src2).
  - alpha_dst per edge via a transposed one-hot T (dst-row-major) built
    on DVE from a DMA-broadcast d128T row, then per-chunk matmuls
    T_k @ aD_win -- removes the 256B-per-edge aD gather entirely
    (one third of baseline gather indices and SWDGE descgen time).
  - Layer-2 rows [rot-h2 (64) | pad] bf16 = 256B with a_src2 folded in;
    aD2 via the same T trick.
Gathers per window drop from 3 to 2 (lo/hi src half of the int16 index
space), gather bytes per edge from 1536B+ to 512B (L1) + 256B (L2).
"""
import sys

for _p in ("/opt/trn_rl_repo", "/opt/pypackages"):
    if _p not in sys.path:
        sys.path.insert(0, _p)

import numpy as np
from concourse import bacc, bass, mybir, tile
from concourse.masks import make_identity

P = 128
F32 = mybir.dt.float32
BF16 = mybir.dt.bfloat16
I16 = mybir.dt.int16
HALF = 32768

# ---- problem constants (nn_GAT_60000693125135) ----
N = 50000
IN_DIM = 256
H1 = 8
HID = 32
HC1 = H1 * HID  # 256
OUT = 64
NCORES = 8
NEG_SLOPE = 0.2

SP = False       # dma_gather single_packet
PMUL4D = True    # one 4D DVE op for the per-head p multiply


def _cdiv(a, b):
    return -(-a // b)


def _wrap16(vals, nidx):
    a = np.asarray(vals, np.int16).reshape(nidx // 16, 16).T
    return np.tile(a, (8, 1))


def _bf16(a):
    import ml_dtypes
    return np.asarray(a, np.float32).astype(ml_dtypes.bfloat16)


# ----------------------------------------------------------------------------
# Host-side preprocessing.
# ----------------------------------------------------------------------------
def prep_edges(edge_index, n, ncores):
    """Shard + window + src-half-split the edge list (self loops added).

    Per-core arrays: srclo16/srchi16 (wrap-16 gather idx), d128 [P, CT]
    (dst row in window, -1 on pads), d128T [nw, cmax*128] (transposed
    layout for the T build, -1 on pads)."""
    e0 = edge_index[0].astype(np.int64)
    e1 = edge_index[1].astype(np.int64)
    loops = np.arange(n, dtype=np.int64)
    src = np.concatenate([e0, loops])
    dst = np.concatenate([e1, loops])

    nchunk = n // ncores
    nw = _cdiv(nchunk, P)
    core = dst // nchunk
    dloc = dst - core * nchunk
    w = dloc // P
    hi = (src >= HALF).astype(np.int64)
    gid = (core * nw + w) * 2 + hi
    ngroups = ncores * nw * 2
    cnt = np.bincount(gid, minlength=ngroups).reshape(ncores, nw, 2)
    CWlo = _cdiv(cnt[:, :, 0].max(axis=0), P)
    CWhi = _cdiv(cnt[:, :, 1].max(axis=0), P)
    CW = CWlo + CWhi
    assert CW.min() >= 1
    CTlo, CThi, CT = int(CWlo.sum()), int(CWhi.sum()), int(CW.sum())
    cmax = int(CW.max())

    order = np.argsort(gid, kind="stable")
    gid_s = gid[order]
    starts = np.concatenate([[0], np.cumsum(np.bincount(gid_s, minlength=ngroups))])
    pos = np.arange(order.size) - starts[gid_s]
    c_s = core[order]
    w_s = w[order]
    hi_s = hi[order]
    slot = pos + hi_s * (CWlo[w_s] * P)

    srcv = np.zeros((ncores, nw, cmax * P), np.int64)
    d128v = np.full((ncores, nw, cmax * P), -1.0, np.float32)
    srcv[c_s, w_s, slot] = src[order] - hi_s * HALF
    d128v[c_s, w_s, slot] = (dloc[order] % P).astype(np.float32)

    srclo16 = np.zeros((ncores, P, 8 * CTlo), np.int16)
    srchi16 = np.zeros((ncores, P, 8 * CThi), np.int16)
    d128A = np.full((ncores, P, CT), -1.0, np.float32)
    d128T = np.full((ncores, nw, cmax * P), -1.0, np.float32)
    olo = ohi = oall = 0
    for wi in range(nw):
        nlo, nhi, nall = int(CWlo[wi]) * P, int(CWhi[wi]) * P, int(CW[wi]) * P
        for c in range(ncores):
            if nlo:
                srclo16[c, :, 8 * olo:8 * (olo + nlo // P)] = _wrap16(srcv[c, wi, :nlo], nlo)
            if nhi:
                srchi16[c, :, 8 * ohi:8 * (ohi + nhi // P)] = _wrap16(srcv[c, wi, nlo:nall], nhi)
            d128A[c, :, oall:oall + nall // P] = d128v[c, wi, :nall].reshape(nall // P, P).T
            d128T[c, wi, :nall] = d128v[c, wi, :nall]
        olo += nlo // P
        ohi += nhi // P
        oall += nall // P
    return dict(srclo16=srclo16, srchi16=srchi16, d128=d128A, d128T=d128T,
                CWlo=[int(v) for v in CWlo], CWhi=[int(v) for v in CWhi],
                cmax=cmax)


def _rot(a):
    """Invertible [d, d] block with column 0 == a, rest orthonormal."""
    a = np.asarray(a, np.float64)
    d = a.size
    q, _ = np.linalg.qr(np.column_stack([a, np.eye(d)[:, 1:]]))
    B = q.copy()
    B[:, 0] = a
    s = float(q[:, 0] @ a)
    Binv = q.T.copy()
    Binv[0, :] /= s
    assert np.abs(B @ Binv - np.eye(d)).max() < 1e-9
    return B.astype(np.float64), Binv.astype(np.float64)


# ----------------------------------------------------------------------------
# Kernel builder (SPMD program, same for all cores).
# ----------------------------------------------------------------------------
def build_nc(cfg):
    n = cfg["N"]; in_dim = cfg["IN"]; hc1 = cfg["HC1"]; h1 = cfg["H1"]
    out_dim = cfg["OUT"]; ncores = cfg["NCORES"]; neg = cfg["NEG"]
    CWlo, CWhi = cfg["CWlo"], cfg["CWhi"]
    CW = [a + b for a, b in zip(CWlo, CWhi)]
    cmax = cfg["cmax"]
    b1nz, b2nz = cfg["B1NZ"], cfg["B2NZ"]

    nchunk = n // ncores
    nw = _cdiv(nchunk, P)
    assert len(CW) == nw
    CTlo, CThi, CT = sum(CWlo), sum(CWhi), sum(CW)
    ntiles = _cdiv(n, P)
    npad = ntiles * P
    nlpad = nw * P
    kt1 = _cdiv(in_dim, P)   # 2
    ckt = _cdiv(hc1, P)      # 2
    NB = 8
    W2C = out_dim + 1        # 65: [rot-h2 | aD2]

    CMAXP = cmax * P
    nc = bacc.Bacc(None, target_bir_lowering=False, debug=False,
                   num_devices=ncores, num_swdge_queues=4)

    # ---- I/O ----
    xT_in = nc.dram_tensor("xT", [in_dim, npad], BF16, kind="ExternalInput")
    w1e_in = nc.dram_tensor("W1e", [in_dim, hc1 + h1], BF16, kind="ExternalInput")
    binv_in = nc.dram_tensor("Binv", [hc1, hc1], BF16, kind="ExternalInput")
    w2e_in = nc.dram_tensor("W2e", [hc1, W2C], BF16, kind="ExternalInput")
    m2i_in = nc.dram_tensor("M2i", [out_dim, out_dim], BF16, kind="ExternalInput")
    b1r_in = nc.dram_tensor("b1r", [P, hc1], BF16, kind="ExternalInput")
    b2r_in = nc.dram_tensor("b2r", [P, out_dim], F32, kind="ExternalInput")
    iota_in = nc.dram_tensor("iota", [P, P], BF16, kind="ExternalInput")
    iotac_in = nc.dram_tensor("iotac", [P, 1], F32, kind="ExternalInput")
    ones_in = nc.dram_tensor("ones1", [1, P], BF16, kind="ExternalInput")
    d128_in = nc.dram_tensor("d128", [P, CT], BF16, kind="ExternalInput")
    d128T_in = nc.dram_tensor("d128T", [nw, cmax * P], BF16, kind="ExternalInput")
    slo_in = nc.dram_tensor("srclo16", [P, 8 * CTlo], I16, kind="ExternalInput")
    shi_in = nc.dram_tensor("srchi16", [P, max(8 * CThi, 16)], I16, kind="ExternalInput")
    out_ext = nc.dram_tensor("out", [nchunk, out_dim], F32, kind="ExternalOutput")

    from contextlib import ExitStack
    with tile.TileContext(nc) as tc, ExitStack() as es:
        if True:
            dram = es.enter_context(tc.tile_pool(name="dram", bufs=1, space="DRAM"))
            cpool = es.enter_context(tc.tile_pool(name="const", bufs=1))
            xpool = es.enter_context(tc.tile_pool(name="xst", bufs=2))
            hpool = es.enter_context(tc.tile_pool(name="hst", bufs=2))
            gpool = es.enter_context(tc.tile_pool(name="gbuf", bufs=5))
            gwpool = es.enter_context(tc.tile_pool(name="gw", bufs=2))
            g2pool = es.enter_context(tc.tile_pool(name="g2buf", bufs=4))
            spool = es.enter_context(tc.tile_pool(name="sbuf", bufs=2))
            tpool = es.enter_context(tc.tile_pool(name="tbuf", bufs=2))
            pepool = es.enter_context(tc.tile_pool(name="pebuf", bufs=2))
            o1pool = es.enter_context(tc.tile_pool(name="o1", bufs=2))
            smpool = es.enter_context(tc.tile_pool(name="small", bufs=3))
            psA = es.enter_context(tc.tile_pool(name="psA", bufs=2, space="PSUM"))
            psB = es.enter_context(tc.tile_pool(name="psB", bufs=1, space="PSUM"))
            psD = es.enter_context(tc.tile_pool(name="psD", bufs=2, space="PSUM"))
            psT = es.enter_context(tc.tile_pool(name="psT", bufs=1, space="PSUM"))
            psQ = es.enter_context(tc.tile_pool(name="psQ", bufs=1, space="PSUM"))

            # ---- DRAM scratch ----
            h_tab = dram.tile([npad, hc1], BF16)
            aDfull = dram.tile([npad, h1], BF16)
            h2_mine = dram.tile([nchunk, out_dim], BF16)
            aD2_loc = dram.tile([nlpad, 1], BF16)
            h2c_tab = dram.tile([n, out_dim], BF16, addr_space="Shared")
            h2_tab = dram.tile([n, 2 * out_dim], BF16)

            # ---- resident constants ----
            iota_t = cpool.tile([P, P], BF16)
            nc.sync.dma_start(out=iota_t[:], in_=iota_in[:])
            iotac_t = cpool.tile([P, 1], F32)
            nc.sync.dma_start(out=iotac_t[:], in_=iotac_in[:])
            ones1 = cpool.tile([1, P], BF16)
            nc.sync.dma_start(out=ones1[:], in_=ones_in[:])
            d128_t = cpool.tile([P, CT], BF16)
            nc.sync.dma_start(out=d128_t[:], in_=d128_in[:])
            ident = cpool.tile([P, P], BF16)
            make_identity(nc, ident[:])
            b1r = cpool.tile([P, hc1], BF16)
            nc.sync.dma_start(out=b1r[:], in_=b1r_in[:])
            b2r = cpool.tile([P, out_dim], F32)
            nc.sync.dma_start(out=b2r[:], in_=b2r_in[:])
            slo = cpool.tile([P, 8 * CTlo], I16)
            nc.sync.dma_start(out=slo[:], in_=slo_in[:])
            shi = cpool.tile([P, max(8 * CThi, 16)], I16)
            nc.sync.dma_start(out=shi[:], in_=shi_in[:])
            w1e = cpool.tile([P, kt1, hc1 + h1], BF16)
            for kt in range(kt1):
                nc.sync.dma_start(out=w1e[:, kt, :], in_=w1e_in[kt * P:(kt + 1) * P, :])
            binv = cpool.tile([P, ckt, hc1], BF16)
            for c in range(ckt):
                nc.sync.dma_start(out=binv[:, c, :], in_=binv_in[c * P:(c + 1) * P, :])
            w2e = cpool.tile([P, ckt, W2C], BF16)
            for c in range(ckt):
                nc.sync.dma_start(out=w2e[:, c, :], in_=w2e_in[c * P:(c + 1) * P, :])
            m2i = cpool.tile([out_dim, out_dim], BF16)
            nc.sync.dma_start(out=m2i[:], in_=m2i_in[:])

            # ---- phase 1: h_ext = x @ w1e -> h_tab + aDfull (replicated) ----
            for g in range(_cdiv(ntiles, NB)):
                nt0 = g * NB
                nb = min(NB, ntiles - nt0)
                xst = xpool.tile([P, kt1, NB * P], BF16, tag="xst")
                for kt in range(kt1):
                    nc.sync.dma_start(out=xst[:, kt, 0:nb * P],
                                      in_=xT_in[kt * P:(kt + 1) * P, nt0 * P:(nt0 + nb) * P])
                hstg = hpool.tile([P, NB, hc1 + h1], BF16, tag="hst")
                for j in range(nb):
                    ps = psA.tile([P, hc1 + h1], F32, tag="ops")
                    for kt in range(kt1):
                        nc.tensor.matmul(out=ps[:], lhsT=xst[:, kt, j * P:(j + 1) * P],
                                         rhs=w1e[:, kt, :], start=(kt == 0), stop=(kt == kt1 - 1))
                    nc.scalar.copy(out=hstg[:, j, :], in_=ps[:])
                hv = h_tab[nt0 * P:(nt0 + nb) * P, :].rearrange("(j p) c -> p j c", p=P)
                nc.sync.dma_start(out=hv, in_=hstg[:, 0:nb, 0:hc1])
                av = aDfull[nt0 * P:(nt0 + nb) * P, :].rearrange("(j p) c -> p j c", p=P)
                nc.sync.dma_start(out=av, in_=hstg[:, 0:nb, hc1:])

            pid_rows = nc.sync.snap(nc.sync.partition_id() * nchunk)
            stop = cfg.get("STOP", "")

            def bounce_out(src_dram, width):
                for w in range(nw):
                    rows = min(P, nchunk - w * P)
                    dbg = smpool.tile([P, out_dim], F32, tag="dbg")
                    nc.vector.tensor_scalar(
                        out=dbg[:rows, :],
                        in0=src_dram[w * P:w * P + rows, 0:width],
                        scalar1=1.0, scalar2=None, op0=mybir.AluOpType.mult)
                    nc.sync.dma_start(out=out_ext[w * P:w * P + rows, :],
                                      in_=dbg[:rows, :])

            if stop == "phase1":
                # bounce own-chunk h_tab rows (rotated) for host check
                for w in range(nw):
                    rows = min(P, nchunk - w * P)
                    dbg = smpool.tile([P, out_dim], F32, tag="dbg")
                    src = h_tab[bass.ds(pid_rows + w * P, rows), 0:out_dim]
                    sb = smpool.tile([P, out_dim], BF16, tag="dbgb")
                    nc.sync.dma_start(out=sb[:rows, :], in_=src)
                    nc.vector.tensor_scalar(out=dbg[:rows, :], in0=sb[:rows, :],
                                            scalar1=1.0, scalar2=None,
                                            op0=mybir.AluOpType.mult)
                    nc.sync.dma_start(out=out_ext[w * P:w * P + rows, :],
                                      in_=dbg[:rows, :])
                return nc

            # ---- phase 2: layer-1 edge aggregation per dst window ----
            # software-pipelined: stage A(w+1) is emitted before stage B(w)
            # so each engine always has independent work queued.
            OLO = [0] * nw
            OALL = [0] * nw
            _olo = _oall = 0
            for w in range(nw):
                OLO[w] = _olo; OALL[w] = _oall
                _olo += CWlo[w]; _oall += CW[w]

            def p2_stageA(w):
                Clo, Chi, C = CWlo[w], CWhi[w], CW[w]
                rows = min(P, nchunk - w * P)
                olo, oall = OLO[w], OALL[w]
                G = gpool.tile([P, cmax, hc1], BF16, tag="G")
                if Clo:
                    nc.gpsimd.dma_gather(
                        out_ap=G[:, 0:Clo, :], in_ap=h_tab[0:HALF, :],
                        idxs_ap=slo[:, 8 * olo:8 * (olo + Clo)],
                        num_idxs=Clo * P, num_idxs_reg=Clo * P, elem_size=hc1,
                        single_packet=SP, queue_num=(2 * w) % 4)
                if Chi:
                    nc.gpsimd.dma_gather(
                        out_ap=G[:, Clo:C, :], in_ap=h_tab[HALF:, :],
                        idxs_ap=shi[:, 8 * (oall - olo):8 * (oall - olo + Chi)],
                        num_idxs=Chi * P, num_idxs_reg=Chi * P, elem_size=hc1,
                        single_packet=SP, queue_num=(2 * w + 1) % 4)
                aDw = smpool.tile([P, h1], BF16, tag="aDw")
                nc.sync.dma_start(out=aDw[:rows, :],
                                  in_=aDfull[bass.ds(pid_rows + w * P, rows), :])
                S = spool.tile([P, cmax, P], BF16, tag="S")
                nc.vector.tensor_tensor(
                    out=S[:, 0:C, :],
                    in0=d128_t[:, oall:oall + C].unsqueeze(-1).to_broadcast((P, C, P)),
                    in1=iota_t[:].unsqueeze(1).to_broadcast((P, C, P)),
                    op=mybir.AluOpType.is_equal)
                dT = tpool.tile([1, cmax * P], BF16, tag="dT")
                nc.sync.dma_start(out=dT[:, 0:C * P], in_=d128T_in[w:w + 1, 0:C * P])
                T = tpool.tile([P, cmax, P], BF16, tag="T")
                for g0 in range(0, C, 8):
                    gn = min(8, C - g0)
                    Qps = psQ.tile([P, 8 * P], F32, tag="q")
                    for h0 in range(0, gn, 4):
                        hn = min(4, gn - h0)
                        nc.tensor.matmul(
                            out=Qps[:, h0 * P:(h0 + hn) * P], lhsT=ones1[:],
                            rhs=dT[0:1, (g0 + h0) * P:(g0 + h0 + hn) * P],
                            start=True, stop=True)
                    nc.vector.tensor_tensor(
                        out=T[:, g0:g0 + gn, :],
                        in0=Qps[:, 0:gn * P].rearrange("r (k q) -> r k q", k=gn),
                        in1=iotac_t[:, 0:1].unsqueeze(1).to_broadcast((P, gn, P)),
                        op=mybir.AluOpType.is_equal)
                aDps = psD.tile([P, cmax, h1], F32, tag="aD")
                for k in range(C):
                    nc.tensor.matmul(out=aDps[:, k, :], lhsT=T[:, k, :], rhs=aDw[:],
                                     start=True, stop=True)
                aDsb = smpool.tile([P, cmax, h1], BF16, tag="aDsb")
                nc.scalar.copy(out=aDsb[:, 0:C, :], in_=aDps[:, 0:C, :])
                pe = pepool.tile([P, cmax, h1], BF16, tag="pe")
                G4 = G[:, 0:C, :].rearrange("p c (h j) -> p c h j", h=h1)
                nc.vector.tensor_tensor(
                    out=pe[:, 0:C, :], in0=G4[:, :, :, 0].squeeze(),
                    in1=aDsb[:, 0:C, :], op=mybir.AluOpType.add)
                nc.vector.scalar_tensor_tensor(
                    out=pe[:, 0:C, :], in0=pe[:, 0:C, :], scalar=neg,
                    in1=pe[:, 0:C, :], op0=mybir.AluOpType.mult,
                    op1=mybir.AluOpType.max)
                nc.scalar.activation(out=pe[:, 0:C, :], in_=pe[:, 0:C, :],
                                     func=mybir.ActivationFunctionType.Exp)
                GW = gwpool.tile([P, cmax, hc1 + h1], BF16, tag="GW")
                GW4 = GW[:, 0:C, 0:hc1].rearrange("p c (h j) -> p c h j", h=h1)
                nc.vector.tensor_tensor(
                    out=GW4, in0=G4,
                    in1=pe[:, 0:C, :].unsqueeze(-1).to_broadcast((P, C, h1, HID)),
                    op=mybir.AluOpType.mult)
                nc.scalar.copy(out=GW[:, 0:C, hc1:], in_=pe[:, 0:C, :])
                return S, GW

            def p2_stageB(w, S, GW):
                Clo, Chi, C = CWlo[w], CWhi[w], CW[w]
                rows = min(P, nchunk - w * P)
                ops = psA.tile([P, hc1 + h1], F32, tag="ops")
                for k in range(C):
                    nc.tensor.matmul(out=ops[:], lhsT=S[:, k, :], rhs=GW[:, k, :],
                                     start=(k == 0), stop=(k == C - 1))
                rec = smpool.tile([P, h1], F32, tag="rec")
                nc.vector.reciprocal(out=rec[:], in_=ops[:, hc1:])
                opssb = o1pool.tile([P, hc1], BF16, tag="opssb")
                nc.scalar.copy(out=opssb[:], in_=ops[:, 0:hc1])
                ats = []
                for c in range(ckt):
                    tp = psT.tile([P, P], BF16, tag="tp")
                    nc.tensor.transpose(tp[:], opssb[:, c * P:(c + 1) * P], ident[:])
                    at = o1pool.tile([P, P], BF16, tag="at")
                    nc.scalar.copy(out=at[:], in_=tp[:])
                    ats.append(at)
                h1u = psB.tile([P, hc1], F32, tag="h1u")
                for c in range(ckt):
                    nc.tensor.matmul(out=h1u[:], lhsT=ats[c][:], rhs=binv[:, c, :],
                                     start=(c == 0), stop=(c == ckt - 1))
                h1w = o1pool.tile([P, hc1], BF16, tag="h1w")
                nc.vector.tensor_tensor(
                    out=h1w[:].rearrange("p (h j) -> p h j", h=h1),
                    in0=h1u[:].rearrange("p (h j) -> p h j", h=h1),
                    in1=rec[:].unsqueeze(-1).to_broadcast((P, h1, HID)),
                    op=mybir.AluOpType.mult)
                if b1nz:
                    nc.vector.tensor_tensor(out=h1w[:], in0=h1w[:], in1=b1r[:],
                                            op=mybir.AluOpType.add)
                nc.scalar.activation(out=h1w[:], in_=h1w[:],
                                     func=mybir.ActivationFunctionType.Relu)
                ats2 = []
                for c in range(ckt):
                    tp = psT.tile([P, P], BF16, tag="tp")
                    nc.tensor.transpose(tp[:], h1w[:, c * P:(c + 1) * P], ident[:])
                    at = o1pool.tile([P, P], BF16, tag="at")
                    nc.scalar.copy(out=at[:], in_=tp[:])
                    ats2.append(at)
                h2e = psB.tile([P, W2C], F32, tag="h1u")
                for c in range(ckt):
                    nc.tensor.matmul(out=h2e[:], lhsT=ats2[c][:], rhs=w2e[:, c, :],
                                     start=(c == 0), stop=(c == ckt - 1))
                h2sb = o1pool.tile([P, out_dim], BF16, tag="h2sb")
                nc.scalar.copy(out=h2sb[:], in_=h2e[:, 0:out_dim])
                nc.sync.dma_start(out=h2_mine[w * P:w * P + rows, :], in_=h2sb[:rows, :])
                a2sb = smpool.tile([P, 1], BF16, tag="a2sb")
                nc.scalar.copy(out=a2sb[:], in_=h2e[:, out_dim:out_dim + 1])
                nc.sync.dma_start(out=aD2_loc[w * P:w * P + rows, :], in_=a2sb[:rows, :])

            prev = None
            for w in range(nw):
                cur = p2_stageA(w)
                if prev is not None:
                    p2_stageB(w - 1, *prev)
                prev = cur
            p2_stageB(nw - 1, *prev)

            if stop == "phase2":
                bounce_out(h2_mine, out_dim)
                return nc

            # ---- all-gather h2 ----
            nc.gpsimd.collective_compute(
                "AllGather", mybir.AluOpType.bypass,
                replica_groups=[list(range(ncores))],
                ins=[h2_mine[:].opt()], outs=[h2c_tab[:].opt()])
            # repad 64 -> 128 byte-stride rows for the 256B-min gather
            NRP = 32
            r0 = 0
            while r0 < n:
                nbr = min(NRP, (n - r0) // P)
                if nbr >= 1:
                    rows2 = nbr * P
                    rp = hpool.tile([P, NRP, out_dim], BF16, tag="rp")
                    nc.sync.dma_start(
                        out=rp[:, 0:nbr, :],
                        in_=h2c_tab[r0:r0 + rows2, :].rearrange(
                            "(j p) c -> p j c", p=P))
                    nc.sync.dma_start(
                        out=h2_tab[r0:r0 + rows2, 0:out_dim].rearrange(
                            "(j p) c -> p j c", p=P),
                        in_=rp[:, 0:nbr, :])
                    r0 += rows2
                else:
                    rem = n - r0
                    rp = hpool.tile([P, NRP, out_dim], BF16, tag="rp")
                    nc.sync.dma_start(out=rp[0:rem, 0, :], in_=h2c_tab[r0:n, :])
                    nc.sync.dma_start(out=h2_tab[r0:n, 0:out_dim],
                                      in_=rp[0:rem, 0, :])
                    r0 = n

            if stop == "cc":
                bounce_out(h2_tab, out_dim)
                return nc

            # ---- phase 3: layer-2 edge aggregation + log_softmax ----
            t_all = cpool.tile([P, nw, out_dim], F32)
            s_all = cpool.tile([P, nw], F32)

            def p3_stageA(w):
                Clo, Chi, C = CWlo[w], CWhi[w], CW[w]
                rows = min(P, nchunk - w * P)
                olo, oall = OLO[w], OALL[w]
                G2 = g2pool.tile([P, cmax, 2 * out_dim], BF16, tag="G2")
                if Clo:
                    nc.gpsimd.dma_gather(
                        out_ap=G2[:, 0:Clo, :], in_ap=h2_tab[0:HALF, :],
                        idxs_ap=slo[:, 8 * olo:8 * (olo + Clo)],
                        num_idxs=Clo * P, num_idxs_reg=Clo * P,
                        elem_size=2 * out_dim, single_packet=SP,
                        queue_num=(2 * w) % 4)
                if Chi:
                    nc.gpsimd.dma_gather(
                        out_ap=G2[:, Clo:C, :], in_ap=h2_tab[HALF:, :],
                        idxs_ap=shi[:, 8 * (oall - olo):8 * (oall - olo + Chi)],
                        num_idxs=Chi * P, num_idxs_reg=Chi * P,
                        elem_size=2 * out_dim, single_packet=SP,
                        queue_num=(2 * w + 1) % 4)
                aD2w = smpool.tile([P, 1], BF16, tag="aD2w")
                nc.sync.dma_start(out=aD2w[:rows, :],
                                  in_=aD2_loc[w * P:w * P + rows, :])
                S = spool.tile([P, cmax, P], BF16, tag="S")
                nc.vector.tensor_tensor(
                    out=S[:, 0:C, :],
                    in0=d128_t[:, oall:oall + C].unsqueeze(-1).to_broadcast((P, C, P)),
                    in1=iota_t[:].unsqueeze(1).to_broadcast((P, C, P)),
                    op=mybir.AluOpType.is_equal)
                dT = tpool.tile([1, cmax * P], BF16, tag="dT")
                nc.sync.dma_start(out=dT[:, 0:C * P], in_=d128T_in[w:w + 1, 0:C * P])
                T = tpool.tile([P, cmax, P], BF16, tag="T")
                for g0 in range(0, C, 8):
                    gn = min(8, C - g0)
                    Qps = psQ.tile([P, 8 * P], F32, tag="q")
                    for h0 in range(0, gn, 4):
                        hn = min(4, gn - h0)
                        nc.tensor.matmul(
                            out=Qps[:, h0 * P:(h0 + hn) * P], lhsT=ones1[:],
                            rhs=dT[0:1, (g0 + h0) * P:(g0 + h0 + hn) * P],
                            start=True, stop=True)
                    nc.vector.tensor_tensor(
                        out=T[:, g0:g0 + gn, :],
                        in0=Qps[:, 0:gn * P].rearrange("r (k q) -> r k q", k=gn),
                        in1=iotac_t[:, 0:1].unsqueeze(1).to_broadcast((P, gn, P)),
                        op=mybir.AluOpType.is_equal)
                aD2ps = psD.tile([P, cmax, 1], F32, tag="aD")
                for k in range(C):
                    nc.tensor.matmul(out=aD2ps[:, k, :], lhsT=T[:, k, :], rhs=aD2w[:],
                                     start=True, stop=True)
                aD2sb = smpool.tile([P, cmax], BF16, tag="aD2sb")
                nc.scalar.copy(out=aD2sb[:, 0:C], in_=aD2ps[:, 0:C, 0].squeeze())
                pe2 = pepool.tile([P, cmax], BF16, tag="pe2")
                nc.vector.tensor_tensor(
                    out=pe2[:, 0:C], in0=G2[:, 0:C, 0].squeeze(),
                    in1=aD2sb[:, 0:C], op=mybir.AluOpType.add)
                nc.vector.scalar_tensor_tensor(
                    out=pe2[:, 0:C], in0=pe2[:, 0:C], scalar=neg,
                    in1=pe2[:, 0:C], op0=mybir.AluOpType.mult,
                    op1=mybir.AluOpType.max)
                nc.scalar.activation(out=pe2[:, 0:C], in_=pe2[:, 0:C],
                                     func=mybir.ActivationFunctionType.Exp)
                G2b = pepool.tile([P, cmax, out_dim + 1], BF16, tag="G2b")
                nc.vector.tensor_tensor(
                    out=G2b[:, 0:C, 0:out_dim], in0=G2[:, 0:C, 0:out_dim],
                    in1=pe2[:, 0:C].unsqueeze(-1).to_broadcast((P, C, out_dim)),
                    op=mybir.AluOpType.mult)
                nc.scalar.copy(out=G2b[:, 0:C, out_dim].squeeze(), in_=pe2[:, 0:C])
                return S, G2b

            def p3_stageB(w, S, G2b):
                Clo, Chi, C = CWlo[w], CWhi[w], CW[w]
                rows = min(P, nchunk - w * P)
                ops2 = psA.tile([P, out_dim + 1], F32, tag="ops")
                for k in range(C):
                    nc.tensor.matmul(out=ops2[:], lhsT=S[:, k, :],
                                     rhs=G2b[:, k, :], start=(k == 0), stop=(k == C - 1))
                rec2 = smpool.tile([P, 1], F32, tag="rec2")
                nc.vector.reciprocal(out=rec2[:], in_=ops2[:, out_dim:])
                o2sb = o1pool.tile([P, out_dim], BF16, tag="o2sb")
                nc.scalar.copy(out=o2sb[:], in_=ops2[:, 0:out_dim])
                tp = psT.tile([P, P], BF16, tag="tp")
                nc.tensor.transpose(tp[0:out_dim, :], o2sb[:], ident[:])
                at5 = o1pool.tile([out_dim, P], BF16, tag="at5")
                nc.scalar.copy(out=at5[:], in_=tp[0:out_dim, :])
                z = psB.tile([P, out_dim], F32, tag="h1u")
                nc.tensor.matmul(out=z[:], lhsT=at5[:], rhs=m2i[:],
                                 start=True, stop=True)
                zf = smpool.tile([P, out_dim], F32, tag="zf")
                nc.vector.tensor_tensor(out=zf[:], in0=z[:],
                                        in1=rec2[:].to_broadcast((P, out_dim)),
                                        op=mybir.AluOpType.mult)
                if b2nz:
                    nc.vector.tensor_tensor(out=zf[:], in0=zf[:], in1=b2r[:],
                                            op=mybir.AluOpType.add)
                negmax = smpool.tile([P, 1], F32, tag="negmax")
                nc.vector.tensor_reduce(out=negmax[:], in_=zf[:],
                                        axis=mybir.AxisListType.X,
                                        op=mybir.AluOpType.max, negate=True)
                nc.vector.tensor_tensor(out=t_all[:, w, :], in0=zf[:],
                                        in1=negmax[:].to_broadcast((P, out_dim)),
                                        op=mybir.AluOpType.add)
                esc = smpool.tile([P, out_dim], F32, tag="esc")
                nc.scalar.activation(out=esc[:], in_=t_all[:, w, :],
                                     func=mybir.ActivationFunctionType.Exp,
                                     accum_out=s_all[:, w:w + 1])

            prev3 = None
            for w in range(nw):
                cur = p3_stageA(w)
                if prev3 is not None:
                    p3_stageB(w - 1, *prev3)
                prev3 = cur
            p3_stageB(nw - 1, *prev3)
            lns = cpool.tile([P, nw], F32)
            nc.scalar.activation(out=lns[:], in_=s_all[:],
                                 func=mybir.ActivationFunctionType.Ln)
            for w in range(nw):
                rows = min(P, nchunk - w * P)
                res = smpool.tile([P, out_dim], F32, tag="esc")
                nc.vector.tensor_tensor(out=res[:], in0=t_all[:, w, :],
                                        in1=lns[:, w:w + 1].to_broadcast((P, out_dim)),
                                        op=mybir.AluOpType.subtract)
                nc.sync.dma_start(out=out_ext[w * P:w * P + rows, :], in_=res[:rows, :])

    return nc


# ----------------------------------------------------------------------------
# Host-side input packing.
# ----------------------------------------------------------------------------
def make_in_maps(inputs, cfg):
    n = cfg["N"]; in_dim = cfg["IN"]; hc1 = cfg["HC1"]; h1 = cfg["H1"]
    hid = cfg["HID"]; out_dim = cfg["OUT"]; ncores = cfg["NCORES"]

    x = np.asarray(inputs["x"], np.float32)
    ei = np.asarray(inputs["edge_index"])
    W1 = np.asarray(inputs["W1"], np.float64)
    a_src1 = np.asarray(inputs["a_src1"], np.float64)
    a_dst1 = np.asarray(inputs["a_dst1"], np.float64)
    b1 = np.asarray(inputs["b1"], np.float32)
    W2 = np.asarray(inputs["W2"], np.float64)
    a_src2 = np.asarray(inputs["a_src2"], np.float64)
    a_dst2 = np.asarray(inputs["a_dst2"], np.float64)
    b2 = np.asarray(inputs["b2"], np.float32)

    cfg["B1NZ"] = bool(np.any(b1))
    cfg["B2NZ"] = bool(np.any(b2))

    ntiles = _cdiv(n, P)
    npad = ntiles * P
    xT = np.zeros((in_dim, npad), np.float32)
    xT[:, :n] = x.T

    # rotation blocks: B_h col 0 = a_src1[h]
    W1e = np.zeros((in_dim, hc1 + h1), np.float64)
    Binv = np.zeros((hc1, hc1), np.float64)
    for h in range(h1):
        B, Bi = _rot(a_src1[h])
        W1e[:, h * hid:(h + 1) * hid] = W1[:, h * hid:(h + 1) * hid] @ B
        W1e[:, hc1 + h] = W1[:, h * hid:(h + 1) * hid] @ a_dst1[h]
        Binv[h * hid:(h + 1) * hid, h * hid:(h + 1) * hid] = Bi
    M2, M2i = _rot(a_src2[0])
    W2e = np.zeros((hc1, out_dim + 1), np.float64)
    W2e[:, 0:out_dim] = W2 @ M2
    W2e[:, out_dim] = W2 @ a_dst2[0]

    pe = prep_edges(ei, n, ncores)
    cfg["CWlo"], cfg["CWhi"], cfg["cmax"] = pe["CWlo"], pe["CWhi"], pe["cmax"]

    iota = np.tile(np.arange(P, dtype=np.float32)[None, :], (P, 1))
    common = {
        "xT": _bf16(xT),
        "W1e": _bf16(W1e), "Binv": _bf16(Binv),
        "W2e": _bf16(W2e), "M2i": _bf16(M2i),
        "b1r": _bf16(np.tile(b1[None, :], (P, 1))),
        "b2r": np.tile(b2[None, :], (P, 1)).astype(np.float32),
        "iota": _bf16(iota),
        "iotac": np.arange(P, dtype=np.float32)[:, None],
        "ones1": _bf16(np.ones((1, P), np.float32)),
    }
    in_maps = []
    for c in range(ncores):
        m = dict(common)
        m["srclo16"] = np.ascontiguousarray(pe["srclo16"][c])
        shi = pe["srchi16"][c]
        if shi.shape[1] == 0:
            shi = np.zeros((P, 16), np.int16)
        m["srchi16"] = np.ascontiguousarray(shi)
        m["d128"] = _bf16(pe["d128"][c])
        m["d128T"] = _bf16(pe["d128T"][c])
        in_maps.append(m)
    return in_maps


DEFAULT_CFG = dict(N=N, IN=IN_DIM, HC1=HC1, H1=H1, HID=HID, OUT=OUT,
                   NCORES=NCORES, NEG=NEG_SLOPE)

TRACE = False
LAST_RESULTS = None


def kernel(**inputs) -> np.ndarray:
    global LAST_RESULTS
    from concourse.bass_utils import run_bass_kernel_spmd

    cfg = dict(DEFAULT_CFG)
    in_maps = make_in_maps(inputs, cfg)
    nc = build_nc(cfg)
    if not nc.is_finalized():
        nc.finalize()
    res = run_bass_kernel_spmd(nc, in_maps, core_ids=list(range(cfg["NCORES"])),
                               trace=TRACE)
    LAST_RESULTS = res
    outs = [res.results[c]["out"] for c in range(cfg["NCORES"])]
    return np.concatenate(outs, axis=0).astype(np.float32)
